# revision 2
# baseline (speedup 1.0000x reference)
"""Trainium2 Bass kernel for 3x3 conv (stride 1, pad 1) + bias.

Problem: x (32,128,56,56) f32, weights (256,128,3,3) f32, bias (256,) f32
         -> out (32,256,56,56) f32.

Strategy: data-parallel over batch (4 images per core, 8 cores).
Per core: implicit GEMM. C_in=128 lives on the SBUF partition axis (the
matmul contraction dim). Each image is stored width+height zero-padded
(58x58 grid) in a flat per-image slot so that, for every 3x3 tap (kh,kw),
the conv becomes ONE shifted contiguous matmul over 8 output rows
(N = 8*56 = 448) accumulated in PSUM across the 9 taps. C_out=256 is
split into two 128-partition halves (the matmul M dim). Bias is added
during PSUM->SBUF eviction on the scalar engine.

Head: the PE clock is HAM-gated (4/8 until ~3.4us of sustained activity).
A short junk-matmul warmup (dependent only on a gpsimd memset) starts the
activity window ASAP; weights are split into two tiles (kh=1 taps first)
and the first x piece is small, so real matmuls start as soon as possible
and burn the rest of the cold window doing useful work.

Taps that read the zero pad rows (kh=0 on the first row-chunk, kh=2 on
the last) are restricted to the rows that need them, saving PE cycles.

Tail: evictions land in a per-image-half SBUF staging tile; one big DMA
per image-half (8 total) instead of one per chunk, except the final
image-half which keeps per-chunk DMAs (last one on the idle sync queue)
to shorten the end-of-kernel chain.
"""

import os
from contextlib import ExitStack

import ml_dtypes
import numpy as np

import concourse.bacc as bacc
import concourse.bass as bass
import concourse.mybir as mybir
import concourse.tile as tile
import concourse.bass_utils as bass_utils

N_CORES = 8
B, CIN, H, W = 32, 128, 56, 56
COUT = 256
BPC = B // N_CORES          # images per core
PW, PH = W + 1, H + 2       # grid 58 rows x 57 cols: one shared pad col
GRID = PW * PH              # 3306  (col 0 of each row is the zero pad;
                            #  col 57 === next row's col 0)
SLOT = GRID + 2             # +2 zero guard for the last row's col-57 read
RPC = 8                     # output rows per PSUM chunk
NCHUNK = H // RPC           # 7
NFREE = RPC * W             # 448 moving-dim elements per matmul (2D AP)
KK = 9                      # 3x3 taps

P1 = 1032                   # first x piece: covers grid rows 0..18

DT = mybir.dt.bfloat16
NPDT = ml_dtypes.bfloat16

_CACHE: dict = {}


def _build():
    """Build the per-core Bass program (same program on all 8 cores)."""
    nc = bacc.Bacc("TRN2", target_bir_lowering=False, debug=False,
                   num_devices=N_CORES)
    f32 = mybir.dt.float32
    xp = nc.dram_tensor("xp", [BPC, CIN, SLOT], DT, kind="ExternalInput").ap()
    # wa: kh=1 taps (3,4,5); wb: taps (0,1,2,6,7,8) in that slot order
    wa = nc.dram_tensor("wa", [CIN, 3 * COUT], DT, kind="ExternalInput").ap()
    wb = nc.dram_tensor("wb", [CIN, 6 * COUT], DT, kind="ExternalInput").ap()
    b2 = nc.dram_tensor("b2", [128, 2], f32, kind="ExternalInput").ap()
    out = nc.dram_tensor("out", [BPC, COUT, H, W], f32,
                         kind="ExternalOutput").ap()

    with tile.TileContext(nc) as tc, ExitStack() as ctx:
        const_pool = ctx.enter_context(tc.tile_pool(name="const", bufs=1))
        xpool = ctx.enter_context(tc.tile_pool(name="xp_pool", bufs=1))
        epool = ctx.enter_context(tc.tile_pool(name="epool", bufs=2))
        tpool = ctx.enter_context(tc.tile_pool(name="tpool", bufs=3))
        psum = ctx.enter_context(
            tc.tile_pool(name="psum", bufs=7, space="PSUM"))
        wupool = ctx.enter_context(
            tc.tile_pool(name="wupool", bufs=1, space="PSUM"))

        wbufA = const_pool.tile([CIN, 3 * COUT], DT)
        wbufB = const_pool.tile([CIN, 6 * COUT], DT)
        xbuf = xpool.tile([CIN, BPC * SLOT], DT)
        bbuf = const_pool.tile([128, 2], f32)

        # HAM warmup: junk matmuls gated only on a gpsimd memset, so the
        # PE activity window opens while the input DMAs are in flight.
        wrm = const_pool.tile([128, 512], DT)
        nc.gpsimd.memset(wrm[:], 0)
        wps = wupool.tile([128, 512], f32)
        for _ in range(3):
            nc.tensor.matmul(wps[:], wrm[:, :128], wrm[:],
                             start=True, stop=True)
        nc.tensor.matmul(wps[:, :128], wrm[:, :128], wrm[:, :128],
                         start=True, stop=True)

        # DMA-in. scalar queue: first x piece then the rest of image 0.
        # sync queue: weights (kh=1 tile first), bias, then images 1-3.
        nc.scalar.dma_start(xbuf[:, :P1], xp[0][:, :P1])
        nc.scalar.dma_start(xbuf[:, P1:2 * P1], xp[0][:, P1:2 * P1])
        nc.sync.dma_start(wbufA[:], wa)
        nc.sync.dma_start(wbufB[:], wb)
        nc.sync.dma_start(bbuf[:], b2)
        nc.scalar.dma_start(xbuf[:, 2 * P1:SLOT], xp[0][:, 2 * P1:SLOT])
        hs = SLOT // 2
        for n in range(1, BPC):
            for lo, hi in ((0, hs), (hs, SLOT)):
                nc.sync.dma_start(
                    xbuf[:, n * SLOT + lo:n * SLOT + hi],
                    xp[n][:, lo:hi])

        def wsl(k, h):
            if k in (3, 4, 5):
                c0 = (k - 3) * COUT + h * 128
                return wbufA[:, c0:c0 + 128]
            slot = k if k < 3 else k - 3
            c0 = slot * COUT + h * 128
            return wbufB[:, c0:c0 + 128]

        pss = [psum.tile([128, NFREE], f32, name=f"ps{i}", tag=f"ps{i}",
                         bufs=1)
               for i in range(NCHUNK)]

        def group(n, h, c):
            """Accumulate the 9 (pad-row-restricted) taps for one chunk."""
            ps = pss[c]
            # kh=1 taps first (full range, start), restricted kh last
            order = ((3, 4, 5, 0, 1, 2, 6, 7, 8) if c == NCHUNK - 1
                     else (3, 4, 5, 6, 7, 8, 0, 1, 2))
            for i, k in enumerate(order):
                kh, kw = divmod(k, 3)
                if c == 0 and kh == 0:
                    # out row 0's kh=0 tap reads the zero pad row: skip it
                    rows, dest = RPC - 1, ps[:, W:]
                    s = n * SLOT + PW * 1 + kw
                elif c == NCHUNK - 1 and kh == 2:
                    # out row 55's kh=2 tap reads the zero pad row
                    rows, dest = RPC - 1, ps[:, :7 * W]
                    s = n * SLOT + PW * (RPC * c + kh) + kw
                else:
                    rows, dest = RPC, ps[:]
                    s = n * SLOT + PW * (RPC * c + kh) + kw
                rhs = xbuf[:, s:s + rows * PW].rearrange(
                    "p (r c) -> p r c", c=PW)[:, :, :W]
                nc.tensor.matmul(dest, wsl(k, h), rhs,
                                 start=(i == 0), stop=(i == KK - 1))
            return ps

        for n in range(BPC):
            for h in range(2):
                last_half = (n == BPC - 1 and h == 1)
                # first image-half starts with chunk 1 (all-full taps,
                # needs only the first x piece + the kh=1 weight tile)
                corder = ((1, 0, 2, 3, 4, 5, 6) if (n == 0 and h == 0)
                          else tuple(range(NCHUNK)))
                if not last_half:
                    evh = epool.tile([128, NCHUNK * NFREE], f32)
                    for c in corder:
                        ps = group(n, h, c)
                        nc.scalar.activation(
                            evh[:, c * NFREE:(c + 1) * NFREE], ps[:],
                            mybir.ActivationFunctionType.Identity,
                            bias=bbuf[:, h:h + 1])
                    od = out[n, h * 128:(h + 1) * 128].rearrange(
                        "c r w -> c (r w)")
                    nc.scalar.dma_start(od, evh[:])
                else:
                    # final image-half: per-chunk evictions + DMAs so the
                    # end-of-kernel chain is short; last DMA on sync queue
                    for c in corder:
                        ps = group(n, h, c)
                        ev = tpool.tile([128, NFREE], f32)
                        nc.scalar.activation(
                            ev[:], ps[:],
                            mybir.ActivationFunctionType.Identity,
                            bias=bbuf[:, h:h + 1])
                        od = out[n, h * 128:(h + 1) * 128,
                                 c * RPC:(c + 1) * RPC].rearrange(
                                     "c r w -> c (r w)")
                        if c == NCHUNK - 1:
                            nc.sync.dma_start(od, ev[:])
                        else:
                            nc.scalar.dma_start(od, ev[:])
    nc.compile()
    return nc


def _prep(x, weights, bias):
    """Host-side reshape/pad/cast into the device layouts."""
    xpad = np.zeros((B, CIN, SLOT), dtype=NPDT)
    grid = xpad[:, :, :GRID].reshape(B, CIN, PH, PW)
    # rows 1..56 hold the image; col 0 is the zero pad column (col 57 of a
    # row aliases the next row's col 0, so one pad column serves both edges)
    grid[:, :, 1:1 + H, 1:1 + W] = np.asarray(x).astype(NPDT)
    # weights (co, ci, kh, kw) -> (ci, kh*kw*co) flat, split A (kh=1) / B
    wt = np.ascontiguousarray(
        np.asarray(weights).transpose(1, 2, 3, 0)).reshape(
            CIN, KK * COUT).astype(NPDT)
    wtA = np.ascontiguousarray(wt[:, 3 * COUT:6 * COUT])
    wtB = np.ascontiguousarray(
        np.concatenate([wt[:, :3 * COUT], wt[:, 6 * COUT:]], axis=1))
    b2 = np.ascontiguousarray(
        np.asarray(bias).astype(np.float32).reshape(2, 128).T)
    return xpad, wtA, wtB, b2


def kernel(x, weights, bias):
    if "nc" not in _CACHE:
        _CACHE["nc"] = _build()
    nc = _CACHE["nc"]
    xpad, wtA, wtB, b2 = _prep(x, weights, bias)
    in_maps = [
        {"xp": xpad[i * BPC:(i + 1) * BPC], "wa": wtA, "wb": wtB, "b2": b2}
        for i in range(N_CORES)
    ]
    res = bass_utils.run_bass_kernel_spmd(
        nc, in_maps, core_ids=list(range(N_CORES)),
        trace=bool(int(os.environ.get("CONV_TRACE", "0"))),
    )
    if os.environ.get("CONV_TRACE"):
        _CACHE["last_result"] = res
    return np.concatenate([r["out"] for r in res.results], axis=0)
